# revision 11
# baseline (speedup 1.0000x reference)
"""AttentionBlock3D (GroupNorm + MHA + out-proj + residual) on 8 Trainium2 cores.

The reference contains a deliberate permute quirk ("faithful to original"):
the attention output o[B, nh, N, hd] is reshaped via transpose(1, 2, 0, 3)
-> [nh, N, B, hd] -> view as [B, C, N] before the out-projection.  Deriving
the index bijection: out-proj input "channel" c' = (h%4)*128 + n//32, its
"batch" b' = h//4, and its "token" n' = (n%32)*128 + b*64 + d.

Sharding: core c in {0..3} computes attention for batch 0, heads {2c, 2c+1};
cores {4..7} for batch 1.  Per core: GroupNorm over its batch (replicated),
qkv projection for its 2 heads (fp32r matmuls), attention over all 4096
tokens, PE-transpose of the per-head outputs to token-major (which makes the
DRAM bounce buffer's flat layout exactly the scrambled out-proj input), an
AllGather over groups {0,1,4,5} / {2,3,6,7} (= output-batch b' groups), and
the out-projection sharded by output-channel group + residual.

Attention is computed transposed (keys on partitions): S^T = K_chunk @ Q^T,
P^T = exp(S^T/8) via ScalarE (no max subtraction -- scores are O(1)), and
O^T accumulated with lhsT = [V_chunk | ones] so the softmax denominator
falls out as PSUM row 64; normalization happens after the PE transpose in
token-major layout where the denominator is a per-partition scalar.  The
V-projection bias is folded into the out-proj bias on the host (softmax
rows sum to 1).  All big matmuls run in float32r (TF32-like, 4x faster
than fp32 on the PE, ~1.4e-4 relative error).
"""

import sys

sys.path.insert(0, "/opt/trn_rl_repo")

import numpy as np

import concourse.tile as tile
from concourse import bacc, mybir
from concourse.bass_utils import run_bass_kernel_spmd
from neuron_dtypes._impl import fp32r as _fp32r_impl

B, C, T, H, W = 2, 512, 4, 32, 32
N = T * H * W            # 4096 tokens
NH = 8                   # heads
HD = C // NH             # 64
GROUPS = 32
EPS = 1e-5
NCORES = 8
HPC = 2                  # heads per core
KC = 32                  # key chunks of 128
KG = 3                   # key chunks per exp group (3 PSUM banks)

F32 = mybir.dt.float32
F32R = mybir.dt.float32r
AF = mybir.ActivationFunctionType
ALU = mybir.AluOpType

# AllGather groups = output-batch groups (core order defines row order)
AG_GROUPS = [[0, 1, 4, 5], [2, 3, 6, 7]]


def _round_fp32r(a: np.ndarray) -> np.ndarray:
    flat = np.ascontiguousarray(a, np.float32).view(np.uint32).ravel()
    r = _fp32r_impl.cast_fp32_to_fp32r(flat.size, flat)
    return np.asarray(r, np.uint32).reshape(a.shape).view(np.float32)


def build_nc():
    nc = bacc.Bacc(None, target_bir_lowering=False, debug=False,
                   num_devices=NCORES)

    xb = nc.dram_tensor("xb", [C, N], F32, kind="ExternalInput")
    gnw = nc.dram_tensor("gnw", [C], F32, kind="ExternalInput")
    gnb = nc.dram_tensor("gnb", [C], F32, kind="ExternalInput")
    maskc = nc.dram_tensor("maskc", [128, 4, 32], F32, kind="ExternalInput")
    mask32 = nc.dram_tensor("mask32", [32, C], F32, kind="ExternalInput")
    wq = nc.dram_tensor("wq", [C, 128], F32R, kind="ExternalInput")
    wk = nc.dram_tensor("wk", [C, 128], F32R, kind="ExternalInput")
    wv = nc.dram_tensor("wv", [C, 256], F32R, kind="ExternalInput")
    wo = nc.dram_tensor("wo", [C, 128], F32R, kind="ExternalInput")
    qb = nc.dram_tensor("qb", [128], F32, kind="ExternalInput")
    kb = nc.dram_tensor("kb", [128], F32, kind="ExternalInput")
    id66 = nc.dram_tensor("id66", [66, 66], F32R, kind="ExternalInput")
    obx = nc.dram_tensor("obx", [128, 128], F32, kind="ExternalInput")
    xres = nc.dram_tensor("xres", [128, N], F32, kind="ExternalInput")
    y = nc.dram_tensor("y", [128, N], F32, kind="ExternalOutput")

    with tile.TileContext(nc) as tc:
        _body(nc, tc, xb, gnw, gnb, maskc, mask32, wq, wk, wv, wo,
              qb, kb, id66, obx, xres, y)
    nc.compile()
    return nc


def _body(nc, tc, xb, gnw, gnb, maskc, mask32, wq, wk, wv, wo,
          qb, kb, id66, obx, xres, y):
    with (
        tc.tile_pool(name="const", bufs=1) as const,
        tc.tile_pool(name="dram", bufs=1, space="DRAM") as dram,
    ):
        # ---- constants / weights (DMA'd up front, overlap with GN) ----
        wq_sb = const.tile([128, 4, 128], F32R)
        wk_sb = const.tile([128, 4, 128], F32R)
        wv_sb = const.tile([128, 4, 256], F32R)
        wo_sb = const.tile([128, 4, 128], F32R)
        nc.sync.dma_start(out=wq_sb, in_=wq.rearrange("(c p) m -> p c m", p=128))
        nc.sync.dma_start(out=wk_sb, in_=wk.rearrange("(c p) m -> p c m", p=128))
        nc.sync.dma_start(out=wv_sb, in_=wv.rearrange("(c p) m -> p c m", p=128))
        nc.sync.dma_start(out=wo_sb, in_=wo.rearrange("(c p) m -> p c m", p=128))
        maskc_sb = const.tile([128, 4, 32], F32)
        mask32_sb = const.tile([32, C], F32)
        nc.sync.dma_start(out=maskc_sb, in_=maskc[:])
        nc.sync.dma_start(out=mask32_sb, in_=mask32[:])
        gnw_sb = const.tile([128, 4], F32)
        gnb_sb = const.tile([128, 4], F32)
        nc.sync.dma_start(out=gnw_sb, in_=gnw.rearrange("(t p) -> p t", p=128))
        nc.sync.dma_start(out=gnb_sb, in_=gnb.rearrange("(t p) -> p t", p=128))
        qb_sb = const.tile([128, 1], F32)
        kb_sb = const.tile([128, 1], F32)
        nc.sync.dma_start(out=qb_sb, in_=qb[:].unsqueeze(1))
        nc.sync.dma_start(out=kb_sb, in_=kb[:].unsqueeze(1))
        id66_sb = const.tile([66, 66], F32R)
        obx_sb = const.tile([128, 128], F32)
        nc.sync.dma_start(out=id66_sb, in_=id66[:])
        nc.sync.dma_start(out=obx_sb, in_=obx[:])
        eps_sb = const.tile([32, 1], F32)
        nc.vector.memset(eps_sb, EPS)
        zo_sb = const.tile([128, 2], F32)
        nc.vector.memset(zo_sb[:, 0:1], 1.0)
        nc.vector.memset(zo_sb[:, 1:2], 0.0)

        # DRAM bounce for the collective: flat layout of bounce_in is exactly
        # this core's 256 rows of the scrambled out-proj input.
        bounce_in = dram.tile([HPC, N, HD], F32R)
        bounce_out = dram.tile([4, HPC, N, HD], F32R)

        with tc.tile_pool(name="h", bufs=1) as h_pool:
            h_sb = [h_pool.tile([128, N], F32R, tag=f"h{t}", name=f"h{t}")
                    for t in range(4)]

            # ================= Phase A: GroupNorm =================
            with (
                tc.tile_pool(name="x", bufs=1) as x_pool,
                tc.tile_pool(name="gn_tmp", bufs=4) as gnt,
                tc.tile_pool(name="gn_ps", bufs=2, space="PSUM") as gn_ps,
            ):
                x_sb = []
                mv2 = []
                for t in range(4):
                    xt = x_pool.tile([128, N], F32, tag=f"x{t}")
                    nc.sync.dma_start(out=xt, in_=xb[128 * t:128 * (t + 1), :])
                    x_sb.append(xt)
                    stats = gnt.tile([128, 8, 6], F32, tag="stats")
                    for j in range(8):
                        nc.vector.bn_stats(out=stats[:, j, :],
                                           in_=xt[:, 512 * j:512 * (j + 1)])
                    mv = gnt.tile([128, 2], F32, tag="mv")
                    nc.vector.bn_aggr(out=mv, in_=stats)
                    # mv2 = (mean, E[x^2]) per channel
                    m2 = gnt.tile([128, 2], F32, tag=f"m2_{t}")
                    nc.vector.tensor_mul(m2[:, 0:1], mv[:, 0:1], mv[:, 0:1])
                    nc.vector.tensor_add(m2[:, 1:2], mv[:, 1:2], m2[:, 0:1])
                    nc.vector.tensor_copy(m2[:, 0:1], mv[:, 0:1])
                    mv2.append(m2)

                ps32 = gn_ps.tile([32, 2], F32, tag="ps32")
                for t in range(4):
                    nc.tensor.matmul(ps32, maskc_sb[:, t, :], mv2[t],
                                     start=(t == 0), stop=(t == 3))
                # group stats: (mean_g, rstd_g)  [32, 2]
                g_sb = gnt.tile([32, 2], F32, tag="g")
                ms = gnt.tile([32, 2], F32, tag="ms")
                nc.scalar.mul(ms, ps32, 1.0 / 16.0)       # (mean_g, E2_g)
                var = gnt.tile([32, 1], F32, tag="var")
                nc.vector.tensor_mul(var, ms[:, 0:1], ms[:, 0:1])
                nc.vector.tensor_sub(var, ms[:, 1:2], var)
                sd = gnt.tile([32, 1], F32, tag="sd")
                nc.scalar.activation(sd, var, AF.Sqrt, bias=eps_sb)
                nc.vector.tensor_copy(g_sb[:, 0:1], ms[:, 0:1])
                nc.vector.reciprocal(g_sb[:, 1:2], sd)

                for t in range(4):
                    psbc = gn_ps.tile([128, 2], F32, tag="psbc")
                    nc.tensor.matmul(psbc, mask32_sb[:, 128 * t:128 * (t + 1)],
                                     g_sb, start=True, stop=True)
                    sc = gnt.tile([128, 1], F32, tag="sc")
                    sh = gnt.tile([128, 1], F32, tag="sh")
                    nc.vector.tensor_mul(sc, psbc[:, 1:2], gnw_sb[:, t:t + 1])
                    nc.vector.tensor_mul(sh, psbc[:, 0:1], sc)
                    nc.vector.tensor_sub(sh, gnb_sb[:, t:t + 1], sh)
                    nc.vector.tensor_scalar(out=h_sb[t], in0=x_sb[t],
                                            scalar1=sc, scalar2=sh,
                                            op0=ALU.mult, op1=ALU.add)

            # ================= Phase B: qkv projections =================
            with tc.tile_pool(name="kqv", bufs=1) as kqv:
                k_sb = kqv.tile([128, N], F32R, tag="k")
                q_sb = kqv.tile([128, N], F32R, tag="q")
                v_sb = [kqv.tile([128, HPC, HD + 2], F32R, tag=f"v{i}",
                                 name=f"v{i}")
                        for i in range(KC)]

                pps_cm = tc.tile_pool(name="proj_ps", bufs=2, space="PSUM")
                pps = pps_cm.__enter__()
                for t in range(8):
                    psq = pps.tile([128, 512], F32, tag="psq")
                    for cc in range(4):
                        nc.tensor.matmul(psq, wq_sb[:, cc, :],
                                         h_sb[cc][:, 512 * t:512 * (t + 1)],
                                         start=(cc == 0), stop=(cc == 3))
                    nc.vector.tensor_scalar(out=q_sb[:, 512 * t:512 * (t + 1)],
                                            in0=psq, scalar1=qb_sb,
                                            scalar2=None, op0=ALU.add)
                    psk = pps.tile([128, 512], F32, tag="psk")
                    for cc in range(4):
                        nc.tensor.matmul(psk, wk_sb[:, cc, :],
                                         h_sb[cc][:, 512 * t:512 * (t + 1)],
                                         start=(cc == 0), stop=(cc == 3))
                    nc.vector.tensor_scalar(out=k_sb[:, 512 * t:512 * (t + 1)],
                                            in0=psk, scalar1=kb_sb,
                                            scalar2=None, op0=ALU.add)

                for kt in range(KC):
                    psv = pps.tile([128, 256], F32, tag="psv")
                    for cc in range(4):
                        nc.tensor.matmul(psv,
                                         h_sb[cc][:, 128 * kt:128 * (kt + 1)],
                                         wv_sb[:, cc, :],
                                         start=(cc == 0), stop=(cc == 3))
                    # psv cols 0:64 head0, 64:128 head1 -> v_sb[kt][:, hi, 0:64]
                    nc.scalar.copy(
                        out=v_sb[kt][:, :, 0:HD],
                        in_=psv[:, 0:128].rearrange("p (h d) -> p h d", h=HPC))
                    for hi in range(HPC):
                        nc.vector.tensor_copy(v_sb[kt][:, hi, HD:HD + 2],
                                              zo_sb)
                pps_cm.__exit__(None, None, None)

                with tc.tile_pool(name="so", bufs=1) as so_pool:
                    so_sb = [so_pool.tile([HD + 2, 512], F32R,
                                          tag=f"so{i}", name=f"so{i}")
                             for i in range(HPC * 8)]

                    # ================= Phase C: attention =================
                    with (
                        tc.tile_pool(name="p_sb", bufs=2) as p_pool,
                        tc.tile_pool(name="s_ps", bufs=2, space="PSUM") as s_ps,
                        tc.tile_pool(name="o_ps", bufs=2, space="PSUM") as o_ps,
                    ):
                        groups = [(g * KG, min(KG, KC - g * KG))
                                  for g in range((KC + KG - 1) // KG)]
                        for hi in range(HPC):
                            hofs = HD * hi
                            for jq in range(8):
                                qsl = q_sb[hofs:hofs + HD,
                                           512 * jq:512 * (jq + 1)]
                                pso = o_ps.tile([HD + 2, 512], F32, tag="pso")
                                pending = None  # PV delayed one group so the
                                # PE FIFO never queues a PV (gated on ACT)
                                # ahead of the next S-group.
                                for g0, glen in groups:
                                    pss = s_ps.tile([128, KG * 512], F32,
                                                    tag="pss")
                                    for i in range(glen):
                                        kt = g0 + i
                                        nc.tensor.matmul(
                                            pss[:, 512 * i:512 * (i + 1)],
                                            k_sb[hofs:hofs + HD,
                                                 128 * kt:128 * (kt + 1)],
                                            qsl, start=True, stop=True)
                                    pt = p_pool.tile([128, KG * 512], F32R,
                                                     tag="pt")
                                    nc.scalar.activation(
                                        pt[:, 0:512 * glen],
                                        pss[:, 0:512 * glen],
                                        AF.Exp, scale=float(1.0 / np.sqrt(HD)))
                                    if pending is not None:
                                        pg0, pglen, ppt = pending
                                        for i in range(pglen):
                                            kt = pg0 + i
                                            nc.tensor.matmul(
                                                pso, v_sb[kt][:, hi, :],
                                                ppt[:, 512 * i:512 * (i + 1)],
                                                start=(kt == 0),
                                                stop=(kt == KC - 1))
                                    pending = (g0, glen, pt)
                                pg0, pglen, ppt = pending
                                for i in range(pglen):
                                    kt = pg0 + i
                                    nc.tensor.matmul(
                                        pso, v_sb[kt][:, hi, :],
                                        ppt[:, 512 * i:512 * (i + 1)],
                                        start=(kt == 0), stop=(kt == KC - 1))
                                nc.vector.tensor_copy(so_sb[8 * hi + jq], pso)

                    # ====== Phase C2: transpose to token-major + normalize ===
                    with (
                        tc.tile_pool(name="otok", bufs=3) as otok_pool,
                        tc.tile_pool(name="c2_tmp", bufs=8) as c2t,
                        tc.tile_pool(name="t_ps", bufs=4, space="PSUM") as t_ps,
                    ):
                        for hi in range(HPC):
                            for jq in range(8):
                                so = so_sb[8 * hi + jq]
                                ot = otok_pool.tile([128, 4, HD], F32R,
                                                    tag="ot")
                                for cpos in range(4):
                                    tps = t_ps.tile([128, HD + 2], F32R,
                                                    tag="tps")
                                    nc.tensor.transpose(
                                        tps, so[:, 128 * cpos:128 * (cpos + 1)],
                                        id66_sb)
                                    recip = c2t.tile([128, 1], F32, tag="recip")
                                    nc.vector.reciprocal(recip,
                                                         tps[:, HD:HD + 1])
                                    nc.vector.tensor_scalar(
                                        out=ot[:, cpos, :],
                                        in0=tps[:, 0:HD],
                                        scalar1=recip, scalar2=None,
                                        op0=ALU.mult)
                                nc.sync.dma_start(
                                    out=bounce_in[hi,
                                                  512 * jq:512 * (jq + 1), :]
                                    .rearrange("(c r) d -> r c d", c=4),
                                    in_=ot)

        # ============ Phase D: AllGather + scrambled out projection ==========
        nc.gpsimd.collective_compute(
            "AllGather", ALU.bypass,
            replica_groups=AG_GROUPS,
            ins=[bounce_in.opt()],
            outs=[bounce_out.opt()],
        )
        with (
            tc.tile_pool(name="og", bufs=3) as og_pool,
            tc.tile_pool(name="fin", bufs=4) as fin,
            tc.tile_pool(name="xr", bufs=1) as xr_pool,
            tc.tile_pool(name="d_ps", bufs=4, space="PSUM") as d_ps,
        ):
            xr_sb = xr_pool.tile([128, N], F32)
            nc.sync.dma_start(out=xr_sb, in_=xres[:])
            xr_v = xr_sb.rearrange("p (j q) -> p j q", q=128)
            y_v = y.rearrange("p (j q) -> p j q", q=128)
            for b in range(2):
                # scram chunk cc rows = head (rank-pair 2b + cc//2, local cc%2)
                srcs = [bounce_out[2 * b + cc // 2, cc % 2]
                        .rearrange("(nh5 j) d -> nh5 (j d)", j=32)
                        for cc in range(4)]
                obx_b = obx_sb[:, 64 * b:64 * (b + 1)]
                for tt in range(4):
                    og = og_pool.tile([128, 4, 512], F32R, tag="og")
                    for cc in range(4):
                        nc.sync.dma_start(
                            out=og[:, cc, :],
                            in_=srcs[cc][:, 512 * tt:512 * (tt + 1)])
                    psd = d_ps.tile([128, 512], F32, tag="psd")
                    for cc in range(4):
                        nc.tensor.matmul(psd, wo_sb[:, cc, :], og[:, cc, :],
                                         start=(cc == 0), stop=(cc == 3))
                    psd_v = psd.rearrange("p (j d) -> p j d", d=HD)
                    t1 = fin.tile([128, 8, HD], F32, tag="t1")
                    nc.vector.tensor_add(
                        t1, psd_v,
                        obx_b.unsqueeze(1).broadcast_to((128, 8, HD)))
                    out_sb = fin.tile([128, 8, HD], F32, tag="out")
                    nc.vector.tensor_add(
                        out_sb, t1,
                        xr_v[:, 8 * tt:8 * (tt + 1), 64 * b:64 * (b + 1)])
                    nc.sync.dma_start(
                        out=y_v[:, 8 * tt:8 * (tt + 1), 64 * b:64 * (b + 1)],
                        in_=out_sb)


# =========================== host-side driver ===========================

def prep_in_maps(x, gn_w, gn_b, qkv_w, qkv_b, out_w, out_b):
    """Build the 8 per-core input maps from the full (unsharded) inputs."""
    x = np.asarray(x, np.float32)
    gn_w = np.asarray(gn_w, np.float32)
    gn_b = np.asarray(gn_b, np.float32)
    qkv_w = np.asarray(qkv_w, np.float32)
    qkv_b = np.asarray(qkv_b, np.float32)
    out_w = np.asarray(out_w, np.float32)
    out_b = np.asarray(out_b, np.float32)

    xf = x.reshape(B, C, N)
    maskc = np.zeros((128, 4, 32), np.float32)
    for t in range(4):
        for p in range(128):
            maskc[p, t, 8 * t + p // 16] = 1.0
    mask32 = np.zeros((32, C), np.float32)
    for c in range(C):
        mask32[c // 16, c] = 1.0
    id66 = np.eye(66, dtype=np.float32)

    qkv_wr = qkv_w.reshape(3, NH, HD, C)
    qkv_br = qkv_b.reshape(3, NH, HD)
    vb_full = qkv_br[2]                      # [NH, HD]
    # position of each core inside its AllGather group + group id (= b')
    pos = {}
    grp = {}
    for gi, cores in enumerate(AG_GROUPS):
        for p, cid in enumerate(cores):
            pos[cid] = p
            grp[cid] = gi

    in_maps = []
    for cid in range(NCORES):
        b = cid // 4               # batch this core attends over
        h0 = HPC * (cid % 4)       # first head this core computes
        bg = grp[cid]              # output-batch group for phase D
        p = pos[cid]               # output-channel slice for phase D
        wq_c = qkv_wr[0, h0:h0 + HPC].reshape(128, C).T     # [C, 128]
        wk_c = qkv_wr[1, h0:h0 + HPC].reshape(128, C).T
        wv_c = qkv_wr[2, h0:h0 + HPC].reshape(128, C).T
        wv_pad = np.zeros((C, 256), np.float32)
        wv_pad[:, :128] = wv_c
        oc = slice(128 * p, 128 * (p + 1))
        # obx[ocl, b*64 + d] = out_b[oc] + sum_hm (sum_nh5 w_o[oc, hm*128+nh5])
        #                                  * vb[4*bg + hm, d]
        w_oc = out_w[oc]                                     # [128, 512]
        wsum = w_oc.reshape(128, 4, 128).sum(axis=2)         # [128, 4] per hm
        vbg = vb_full[4 * bg:4 * bg + 4]                     # [4, HD]
        add = wsum @ vbg                                     # [128, HD]
        obx = np.zeros((128, 128), np.float32)
        for bb in range(2):
            obx[:, 64 * bb:64 * (bb + 1)] = out_b[oc][:, None] + add
        in_maps.append({
            "xb": np.ascontiguousarray(xf[b]),
            "gnw": gn_w, "gnb": gn_b,
            "maskc": maskc, "mask32": mask32,
            "wq": _round_fp32r(wq_c),
            "wk": _round_fp32r(wk_c),
            "wv": _round_fp32r(wv_pad),
            "wo": _round_fp32r(w_oc.T.copy()),
            "qb": np.ascontiguousarray(qkv_br[0, h0:h0 + HPC].reshape(128)),
            "kb": np.ascontiguousarray(qkv_br[1, h0:h0 + HPC].reshape(128)),
            "id66": id66,
            "obx": obx,
            "xres": np.ascontiguousarray(xf[bg, oc, :]),
        })
    return in_maps


def assemble_output(results):
    y = np.empty((B, C, N), np.float32)
    for gi, cores in enumerate(AG_GROUPS):
        for p, cid in enumerate(cores):
            y[gi, 128 * p:128 * (p + 1), :] = results[cid]["y"]
    return y.reshape(B, C, T, H, W)


_NC_CACHE = None


def get_nc():
    global _NC_CACHE
    if _NC_CACHE is None:
        _NC_CACHE = build_nc()
    return _NC_CACHE


def kernel(x, gn_w, gn_b, qkv_w, qkv_b, out_w, out_b):
    in_maps = prep_in_maps(x, gn_w, gn_b, qkv_w, qkv_b, out_w, out_b)
    nc = get_nc()
    res = run_bass_kernel_spmd(nc, in_maps, core_ids=list(range(NCORES)))
    out = assemble_output(res.results)
    return out.astype(np.asarray(x).dtype, copy=False)


# revision 12
# speedup vs baseline: 181.3694x; 181.3694x over previous
"""AttentionBlock3D (GroupNorm + MHA + out-proj + residual) on 8 Trainium2 cores.

The reference contains a deliberate permute quirk ("faithful to original"):
the attention output o[B, nh, N, hd] is reshaped via transpose(1, 2, 0, 3)
-> [nh, N, B, hd] -> view as [B, C, N] before the out-projection.  Deriving
the index bijection: out-proj input "channel" c' = (h%4)*128 + n//32, its
"batch" b' = h//4, and its "token" n' = (n%32)*128 + b*64 + d.

Sharding: core c in {0..3} computes attention for batch 0, heads {2c, 2c+1};
cores {4..7} for batch 1.  Per core: GroupNorm over its batch (replicated),
qkv projection for its 2 heads (fp32r matmuls), attention over all 4096
tokens, PE-transpose of the per-head outputs to token-major (which makes the
DRAM bounce buffer's flat layout exactly the scrambled out-proj input), an
AllGather over groups {0,1,4,5} / {2,3,6,7} (= output-batch b' groups), and
the out-projection sharded by output-channel group + residual.

Attention is computed transposed (keys on partitions): S^T = K_chunk @ Q^T,
P^T = exp(S^T/8) via ScalarE (no max subtraction -- scores are O(1)), and
O^T accumulated with lhsT = [V_chunk | ones] so the softmax denominator
falls out as PSUM row 64; normalization happens after the PE transpose in
token-major layout where the denominator is a per-partition scalar.  The
V-projection bias is folded into the out-proj bias on the host (softmax
rows sum to 1).  All big matmuls run in float32r (TF32-like, 4x faster
than fp32 on the PE, ~1.4e-4 relative error).
"""

import sys

sys.path.insert(0, "/opt/trn_rl_repo")

import numpy as np

import concourse.tile as tile
from concourse import bacc, mybir
from concourse.bass_utils import run_bass_kernel_spmd
from neuron_dtypes._impl import fp32r as _fp32r_impl

B, C, T, H, W = 2, 512, 4, 32, 32
N = T * H * W            # 4096 tokens
NH = 8                   # heads
HD = C // NH             # 64
GROUPS = 32
EPS = 1e-5
NCORES = 8
HPC = 2                  # heads per core
KC = 32                  # key chunks of 128
KG = 3                   # key chunks per exp group (3 PSUM banks)

F32 = mybir.dt.float32
F32R = mybir.dt.float32r
AF = mybir.ActivationFunctionType
ALU = mybir.AluOpType

# AllGather groups = output-batch groups (core order defines row order)
AG_GROUPS = [[0, 1, 4, 5], [2, 3, 6, 7]]


def _round_fp32r(a: np.ndarray) -> np.ndarray:
    flat = np.ascontiguousarray(a, np.float32).view(np.uint32).ravel()
    r = _fp32r_impl.cast_fp32_to_fp32r(flat.size, flat)
    return np.asarray(r, np.uint32).reshape(a.shape).view(np.float32)


def build_nc(reps=1):
    nc = bacc.Bacc(None, target_bir_lowering=False, debug=False,
                   num_devices=NCORES)

    xb = nc.dram_tensor("xb", [C, N], F32, kind="ExternalInput")
    gnw = nc.dram_tensor("gnw", [C], F32, kind="ExternalInput")
    gnb = nc.dram_tensor("gnb", [C], F32, kind="ExternalInput")
    maskc = nc.dram_tensor("maskc", [128, 4, 32], F32, kind="ExternalInput")
    mask32 = nc.dram_tensor("mask32", [32, C], F32, kind="ExternalInput")
    wq = nc.dram_tensor("wq", [C, 128], F32R, kind="ExternalInput")
    wk = nc.dram_tensor("wk", [C, 128], F32R, kind="ExternalInput")
    wv = nc.dram_tensor("wv", [C, 256], F32R, kind="ExternalInput")
    wo = nc.dram_tensor("wo", [C, 128], F32R, kind="ExternalInput")
    qb = nc.dram_tensor("qb", [128], F32, kind="ExternalInput")
    kb = nc.dram_tensor("kb", [128], F32, kind="ExternalInput")
    id66 = nc.dram_tensor("id66", [66, 66], F32R, kind="ExternalInput")
    obx = nc.dram_tensor("obx", [128, 128], F32, kind="ExternalInput")
    xres = nc.dram_tensor("xres", [128, N], F32, kind="ExternalInput")
    y = nc.dram_tensor("y", [128, N], F32, kind="ExternalOutput")

    with tile.TileContext(nc) as tc:
        for _ in range(reps):
            _body(nc, tc, xb, gnw, gnb, maskc, mask32, wq, wk, wv, wo,
                  qb, kb, id66, obx, xres, y)
    nc.compile()
    return nc


def _body(nc, tc, xb, gnw, gnb, maskc, mask32, wq, wk, wv, wo,
          qb, kb, id66, obx, xres, y):
    with (
        tc.tile_pool(name="const", bufs=1) as const,
        tc.tile_pool(name="dram", bufs=1, space="DRAM") as dram,
    ):
        # ---- constants / weights (DMA'd up front, overlap with GN) ----
        wq_sb = const.tile([128, 4, 128], F32R)
        wk_sb = const.tile([128, 4, 128], F32R)
        wv_sb = const.tile([128, 4, 256], F32R)
        wo_sb = const.tile([128, 4, 128], F32R)
        nc.sync.dma_start(out=wq_sb, in_=wq.rearrange("(c p) m -> p c m", p=128))
        nc.sync.dma_start(out=wk_sb, in_=wk.rearrange("(c p) m -> p c m", p=128))
        nc.sync.dma_start(out=wv_sb, in_=wv.rearrange("(c p) m -> p c m", p=128))
        nc.sync.dma_start(out=wo_sb, in_=wo.rearrange("(c p) m -> p c m", p=128))
        maskc_sb = const.tile([128, 4, 32], F32)
        mask32_sb = const.tile([32, C], F32)
        nc.sync.dma_start(out=maskc_sb, in_=maskc[:])
        nc.sync.dma_start(out=mask32_sb, in_=mask32[:])
        gnw_sb = const.tile([128, 4], F32)
        gnb_sb = const.tile([128, 4], F32)
        nc.sync.dma_start(out=gnw_sb, in_=gnw.rearrange("(t p) -> p t", p=128))
        nc.sync.dma_start(out=gnb_sb, in_=gnb.rearrange("(t p) -> p t", p=128))
        qb_sb = const.tile([128, 1], F32)
        kb_sb = const.tile([128, 1], F32)
        nc.sync.dma_start(out=qb_sb, in_=qb[:].unsqueeze(1))
        nc.sync.dma_start(out=kb_sb, in_=kb[:].unsqueeze(1))
        id66_sb = const.tile([66, 66], F32R)
        obx_sb = const.tile([128, 128], F32)
        nc.sync.dma_start(out=id66_sb, in_=id66[:])
        nc.sync.dma_start(out=obx_sb, in_=obx[:])
        eps_sb = const.tile([32, 1], F32)
        nc.vector.memset(eps_sb, EPS)
        zo_sb = const.tile([128, 2], F32)
        nc.vector.memset(zo_sb[:, 0:1], 1.0)
        nc.vector.memset(zo_sb[:, 1:2], 0.0)

        # DRAM bounce for the collective: flat layout of bounce_in is exactly
        # this core's 256 rows of the scrambled out-proj input.
        bounce_in = dram.tile([HPC, N, HD], F32R)
        bounce_out = dram.tile([4, HPC, N, HD], F32R)

        with tc.tile_pool(name="h", bufs=1) as h_pool:
            h_sb = [h_pool.tile([128, N], F32R, tag=f"h{t}", name=f"h{t}")
                    for t in range(4)]

            # ================= Phase A: GroupNorm =================
            with (
                tc.tile_pool(name="x", bufs=1) as x_pool,
                tc.tile_pool(name="gn_tmp", bufs=4) as gnt,
                tc.tile_pool(name="gn_ps", bufs=2, space="PSUM") as gn_ps,
            ):
                x_sb = []
                mv2 = []
                for t in range(4):
                    xt = x_pool.tile([128, N], F32, tag=f"x{t}")
                    nc.sync.dma_start(out=xt, in_=xb[128 * t:128 * (t + 1), :])
                    x_sb.append(xt)
                    stats = gnt.tile([128, 8, 6], F32, tag="stats")
                    for j in range(8):
                        nc.vector.bn_stats(out=stats[:, j, :],
                                           in_=xt[:, 512 * j:512 * (j + 1)])
                    mv = gnt.tile([128, 2], F32, tag="mv")
                    nc.vector.bn_aggr(out=mv, in_=stats)
                    # mv2 = (mean, E[x^2]) per channel
                    m2 = gnt.tile([128, 2], F32, tag=f"m2_{t}")
                    nc.vector.tensor_mul(m2[:, 0:1], mv[:, 0:1], mv[:, 0:1])
                    nc.vector.tensor_add(m2[:, 1:2], mv[:, 1:2], m2[:, 0:1])
                    nc.vector.tensor_copy(m2[:, 0:1], mv[:, 0:1])
                    mv2.append(m2)

                ps32 = gn_ps.tile([32, 2], F32, tag="ps32")
                for t in range(4):
                    nc.tensor.matmul(ps32, maskc_sb[:, t, :], mv2[t],
                                     start=(t == 0), stop=(t == 3))
                # group stats: (mean_g, rstd_g)  [32, 2]
                g_sb = gnt.tile([32, 2], F32, tag="g")
                ms = gnt.tile([32, 2], F32, tag="ms")
                nc.scalar.mul(ms, ps32, 1.0 / 16.0)       # (mean_g, E2_g)
                var = gnt.tile([32, 1], F32, tag="var")
                nc.vector.tensor_mul(var, ms[:, 0:1], ms[:, 0:1])
                nc.vector.tensor_sub(var, ms[:, 1:2], var)
                sd = gnt.tile([32, 1], F32, tag="sd")
                nc.scalar.activation(sd, var, AF.Sqrt, bias=eps_sb)
                nc.vector.tensor_copy(g_sb[:, 0:1], ms[:, 0:1])
                nc.vector.reciprocal(g_sb[:, 1:2], sd)

                for t in range(4):
                    psbc = gn_ps.tile([128, 2], F32, tag="psbc")
                    nc.tensor.matmul(psbc, mask32_sb[:, 128 * t:128 * (t + 1)],
                                     g_sb, start=True, stop=True)
                    sc = gnt.tile([128, 1], F32, tag="sc")
                    sh = gnt.tile([128, 1], F32, tag="sh")
                    nc.vector.tensor_mul(sc, psbc[:, 1:2], gnw_sb[:, t:t + 1])
                    nc.vector.tensor_mul(sh, psbc[:, 0:1], sc)
                    nc.vector.tensor_sub(sh, gnb_sb[:, t:t + 1], sh)
                    nc.vector.tensor_scalar(out=h_sb[t], in0=x_sb[t],
                                            scalar1=sc, scalar2=sh,
                                            op0=ALU.mult, op1=ALU.add)

            # ================= Phase B: qkv projections =================
            with tc.tile_pool(name="kqv", bufs=1) as kqv:
                k_sb = kqv.tile([128, N], F32R, tag="k")
                q_sb = kqv.tile([128, N], F32R, tag="q")
                v_sb = [kqv.tile([128, HPC, HD + 2], F32R, tag=f"v{i}",
                                 name=f"v{i}")
                        for i in range(KC)]

                pps_cm = tc.tile_pool(name="proj_ps", bufs=2, space="PSUM")
                pps = pps_cm.__enter__()
                for t in range(8):
                    psq = pps.tile([128, 512], F32, tag="psq")
                    for cc in range(4):
                        nc.tensor.matmul(psq, wq_sb[:, cc, :],
                                         h_sb[cc][:, 512 * t:512 * (t + 1)],
                                         start=(cc == 0), stop=(cc == 3))
                    nc.vector.tensor_scalar(out=q_sb[:, 512 * t:512 * (t + 1)],
                                            in0=psq, scalar1=qb_sb,
                                            scalar2=None, op0=ALU.add)
                    psk = pps.tile([128, 512], F32, tag="psk")
                    for cc in range(4):
                        nc.tensor.matmul(psk, wk_sb[:, cc, :],
                                         h_sb[cc][:, 512 * t:512 * (t + 1)],
                                         start=(cc == 0), stop=(cc == 3))
                    nc.vector.tensor_scalar(out=k_sb[:, 512 * t:512 * (t + 1)],
                                            in0=psk, scalar1=kb_sb,
                                            scalar2=None, op0=ALU.add)

                for kt in range(KC):
                    psv = pps.tile([128, 256], F32, tag="psv")
                    for cc in range(4):
                        nc.tensor.matmul(psv,
                                         h_sb[cc][:, 128 * kt:128 * (kt + 1)],
                                         wv_sb[:, cc, :],
                                         start=(cc == 0), stop=(cc == 3))
                    # psv cols 0:64 head0, 64:128 head1 -> v_sb[kt][:, hi, 0:64]
                    nc.scalar.copy(
                        out=v_sb[kt][:, :, 0:HD],
                        in_=psv[:, 0:128].rearrange("p (h d) -> p h d", h=HPC))
                    for hi in range(HPC):
                        nc.vector.tensor_copy(v_sb[kt][:, hi, HD:HD + 2],
                                              zo_sb)
                pps_cm.__exit__(None, None, None)

                with tc.tile_pool(name="so", bufs=1) as so_pool:
                    so_sb = [so_pool.tile([HD + 2, 512], F32R,
                                          tag=f"so{i}", name=f"so{i}")
                             for i in range(HPC * 8)]

                    # ================= Phase C: attention =================
                    with (
                        tc.tile_pool(name="p_sb", bufs=2) as p_pool,
                        tc.tile_pool(name="s_ps", bufs=2, space="PSUM") as s_ps,
                        tc.tile_pool(name="o_ps", bufs=2, space="PSUM") as o_ps,
                    ):
                        groups = [(g * KG, min(KG, KC - g * KG))
                                  for g in range((KC + KG - 1) // KG)]
                        for hi in range(HPC):
                            hofs = HD * hi
                            for jq in range(8):
                                qsl = q_sb[hofs:hofs + HD,
                                           512 * jq:512 * (jq + 1)]
                                pso = o_ps.tile([HD + 2, 512], F32, tag="pso")
                                pending = None  # PV delayed one group so the
                                # PE FIFO never queues a PV (gated on ACT)
                                # ahead of the next S-group.
                                for g0, glen in groups:
                                    pss = s_ps.tile([128, KG * 512], F32,
                                                    tag="pss")
                                    for i in range(glen):
                                        kt = g0 + i
                                        nc.tensor.matmul(
                                            pss[:, 512 * i:512 * (i + 1)],
                                            k_sb[hofs:hofs + HD,
                                                 128 * kt:128 * (kt + 1)],
                                            qsl, start=True, stop=True)
                                    pt = p_pool.tile([128, KG * 512], F32R,
                                                     tag="pt")
                                    nc.scalar.activation(
                                        pt[:, 0:512 * glen],
                                        pss[:, 0:512 * glen],
                                        AF.Exp, scale=float(1.0 / np.sqrt(HD)))
                                    if pending is not None:
                                        pg0, pglen, ppt = pending
                                        for i in range(pglen):
                                            kt = pg0 + i
                                            nc.tensor.matmul(
                                                pso, v_sb[kt][:, hi, :],
                                                ppt[:, 512 * i:512 * (i + 1)],
                                                start=(kt == 0),
                                                stop=(kt == KC - 1))
                                    pending = (g0, glen, pt)
                                pg0, pglen, ppt = pending
                                for i in range(pglen):
                                    kt = pg0 + i
                                    nc.tensor.matmul(
                                        pso, v_sb[kt][:, hi, :],
                                        ppt[:, 512 * i:512 * (i + 1)],
                                        start=(kt == 0), stop=(kt == KC - 1))
                                nc.vector.tensor_copy(so_sb[8 * hi + jq], pso)

                    # ====== Phase C2: transpose to token-major + normalize ===
                    with (
                        tc.tile_pool(name="otok", bufs=3) as otok_pool,
                        tc.tile_pool(name="c2_tmp", bufs=8) as c2t,
                        tc.tile_pool(name="t_ps", bufs=4, space="PSUM") as t_ps,
                    ):
                        for hi in range(HPC):
                            for jq in range(8):
                                so = so_sb[8 * hi + jq]
                                ot = otok_pool.tile([128, 4, HD], F32R,
                                                    tag="ot")
                                for cpos in range(4):
                                    tps = t_ps.tile([128, HD + 2], F32R,
                                                    tag="tps")
                                    nc.tensor.transpose(
                                        tps, so[:, 128 * cpos:128 * (cpos + 1)],
                                        id66_sb)
                                    recip = c2t.tile([128, 1], F32, tag="recip")
                                    nc.vector.reciprocal(recip,
                                                         tps[:, HD:HD + 1])
                                    nc.vector.tensor_scalar(
                                        out=ot[:, cpos, :],
                                        in0=tps[:, 0:HD],
                                        scalar1=recip, scalar2=None,
                                        op0=ALU.mult)
                                nc.sync.dma_start(
                                    out=bounce_in[hi,
                                                  512 * jq:512 * (jq + 1), :]
                                    .rearrange("(c r) d -> r c d", c=4),
                                    in_=ot)

        # ============ Phase D: AllGather + scrambled out projection ==========
        nc.gpsimd.collective_compute(
            "AllGather", ALU.bypass,
            replica_groups=AG_GROUPS,
            ins=[bounce_in.opt()],
            outs=[bounce_out.opt()],
        )
        with (
            tc.tile_pool(name="og", bufs=3) as og_pool,
            tc.tile_pool(name="fin", bufs=4) as fin,
            tc.tile_pool(name="xr", bufs=1) as xr_pool,
            tc.tile_pool(name="d_ps", bufs=4, space="PSUM") as d_ps,
        ):
            xr_sb = xr_pool.tile([128, N], F32)
            nc.sync.dma_start(out=xr_sb, in_=xres[:])
            xr_v = xr_sb.rearrange("p (j q) -> p j q", q=128)
            y_v = y.rearrange("p (j q) -> p j q", q=128)
            for b in range(2):
                # scram chunk cc rows = head (rank-pair 2b + cc//2, local cc%2)
                srcs = [bounce_out[2 * b + cc // 2, cc % 2]
                        .rearrange("(nh5 j) d -> nh5 (j d)", j=32)
                        for cc in range(4)]
                obx_b = obx_sb[:, 64 * b:64 * (b + 1)]
                for tt in range(4):
                    og = og_pool.tile([128, 4, 512], F32R, tag="og")
                    for cc in range(4):
                        nc.sync.dma_start(
                            out=og[:, cc, :],
                            in_=srcs[cc][:, 512 * tt:512 * (tt + 1)])
                    psd = d_ps.tile([128, 512], F32, tag="psd")
                    for cc in range(4):
                        nc.tensor.matmul(psd, wo_sb[:, cc, :], og[:, cc, :],
                                         start=(cc == 0), stop=(cc == 3))
                    psd_v = psd.rearrange("p (j d) -> p j d", d=HD)
                    t1 = fin.tile([128, 8, HD], F32, tag="t1")
                    nc.vector.tensor_add(
                        t1, psd_v,
                        obx_b.unsqueeze(1).broadcast_to((128, 8, HD)))
                    out_sb = fin.tile([128, 8, HD], F32, tag="out")
                    nc.vector.tensor_add(
                        out_sb, t1,
                        xr_v[:, 8 * tt:8 * (tt + 1), 64 * b:64 * (b + 1)])
                    nc.sync.dma_start(
                        out=y_v[:, 8 * tt:8 * (tt + 1), 64 * b:64 * (b + 1)],
                        in_=out_sb)


# =========================== host-side driver ===========================

def prep_in_maps(x, gn_w, gn_b, qkv_w, qkv_b, out_w, out_b):
    """Build the 8 per-core input maps from the full (unsharded) inputs."""
    x = np.asarray(x, np.float32)
    gn_w = np.asarray(gn_w, np.float32)
    gn_b = np.asarray(gn_b, np.float32)
    qkv_w = np.asarray(qkv_w, np.float32)
    qkv_b = np.asarray(qkv_b, np.float32)
    out_w = np.asarray(out_w, np.float32)
    out_b = np.asarray(out_b, np.float32)

    xf = x.reshape(B, C, N)
    maskc = np.zeros((128, 4, 32), np.float32)
    for t in range(4):
        for p in range(128):
            maskc[p, t, 8 * t + p // 16] = 1.0
    mask32 = np.zeros((32, C), np.float32)
    for c in range(C):
        mask32[c // 16, c] = 1.0
    id66 = np.eye(66, dtype=np.float32)

    qkv_wr = qkv_w.reshape(3, NH, HD, C)
    qkv_br = qkv_b.reshape(3, NH, HD)
    vb_full = qkv_br[2]                      # [NH, HD]
    # position of each core inside its AllGather group + group id (= b')
    pos = {}
    grp = {}
    for gi, cores in enumerate(AG_GROUPS):
        for p, cid in enumerate(cores):
            pos[cid] = p
            grp[cid] = gi

    in_maps = []
    for cid in range(NCORES):
        b = cid // 4               # batch this core attends over
        h0 = HPC * (cid % 4)       # first head this core computes
        bg = grp[cid]              # output-batch group for phase D
        p = pos[cid]               # output-channel slice for phase D
        wq_c = qkv_wr[0, h0:h0 + HPC].reshape(128, C).T     # [C, 128]
        wk_c = qkv_wr[1, h0:h0 + HPC].reshape(128, C).T
        wv_c = qkv_wr[2, h0:h0 + HPC].reshape(128, C).T
        wv_pad = np.zeros((C, 256), np.float32)
        wv_pad[:, :128] = wv_c
        oc = slice(128 * p, 128 * (p + 1))
        # obx[ocl, b*64 + d] = out_b[oc] + sum_hm (sum_nh5 w_o[oc, hm*128+nh5])
        #                                  * vb[4*bg + hm, d]
        w_oc = out_w[oc]                                     # [128, 512]
        wsum = w_oc.reshape(128, 4, 128).sum(axis=2)         # [128, 4] per hm
        vbg = vb_full[4 * bg:4 * bg + 4]                     # [4, HD]
        add = wsum @ vbg                                     # [128, HD]
        obx = np.zeros((128, 128), np.float32)
        for bb in range(2):
            obx[:, 64 * bb:64 * (bb + 1)] = out_b[oc][:, None] + add
        in_maps.append({
            "xb": np.ascontiguousarray(xf[b]),
            "gnw": gn_w, "gnb": gn_b,
            "maskc": maskc, "mask32": mask32,
            "wq": _round_fp32r(wq_c),
            "wk": _round_fp32r(wk_c),
            "wv": _round_fp32r(wv_pad),
            "wo": _round_fp32r(w_oc.T.copy()),
            "qb": np.ascontiguousarray(qkv_br[0, h0:h0 + HPC].reshape(128)),
            "kb": np.ascontiguousarray(qkv_br[1, h0:h0 + HPC].reshape(128)),
            "id66": id66,
            "obx": obx,
            "xres": np.ascontiguousarray(xf[bg, oc, :]),
        })
    return in_maps


def assemble_output(results):
    y = np.empty((B, C, N), np.float32)
    for gi, cores in enumerate(AG_GROUPS):
        for p, cid in enumerate(cores):
            y[gi, 128 * p:128 * (p + 1), :] = results[cid]["y"]
    return y.reshape(B, C, T, H, W)


_NC_CACHE = None


def get_nc():
    global _NC_CACHE
    if _NC_CACHE is None:
        _NC_CACHE = build_nc()
    return _NC_CACHE


def kernel(x, gn_w, gn_b, qkv_w, qkv_b, out_w, out_b):
    in_maps = prep_in_maps(x, gn_w, gn_b, qkv_w, qkv_b, out_w, out_b)
    nc = get_nc()
    res = run_bass_kernel_spmd(nc, in_maps, core_ids=list(range(NCORES)))
    out = assemble_output(res.results)
    return out.astype(np.asarray(x).dtype, copy=False)


# revision 15
# speedup vs baseline: 326.7229x; 1.8014x over previous
"""AttentionBlock3D (GroupNorm + MHA + out-proj + residual) on 8 Trainium2 cores.

The reference contains a deliberate permute quirk ("faithful to original"):
the attention output o[B, nh, N, hd] is reshaped via transpose(1, 2, 0, 3)
-> [nh, N, B, hd] -> view as [B, C, N] before the out-projection.  Deriving
the index bijection: out-proj input "channel" c' = (h%4)*128 + n//32, its
"batch" b' = h//4, and its "token" n' = (n%32)*128 + b*64 + d.

Sharding: core c in {0..3} computes attention for batch 0, heads {2c, 2c+1};
cores {4..7} for batch 1.  Per core: GroupNorm over its batch (replicated),
qkv projection for its 2 heads (fp32r matmuls), attention over all 4096
tokens, PE-transpose of the per-head outputs to token-major (which makes the
DRAM bounce buffer's flat layout exactly the scrambled out-proj input), an
AllGather over groups {0,1,4,5} / {2,3,6,7} (= output-batch b' groups), and
the out-projection sharded by output-channel group + residual.

Attention is computed transposed (keys on partitions): S^T = K_chunk @ Q^T,
P^T = exp(S^T/8) via ScalarE (no max subtraction -- scores are O(1)), and
O^T accumulated with lhsT = [V_chunk | ones] so the softmax denominator
falls out as PSUM row 64; normalization happens after the PE transpose in
token-major layout where the denominator is a per-partition scalar.  The
V-projection bias is folded into the out-proj bias on the host (softmax
rows sum to 1).  All big matmuls run in float32r (TF32-like, 4x faster
than fp32 on the PE, ~1.4e-4 relative error).
"""

import sys

sys.path.insert(0, "/opt/trn_rl_repo")

import numpy as np
import ml_dtypes

import concourse.tile as tile
from concourse import bacc, mybir
from concourse.bass_utils import run_bass_kernel_spmd
from neuron_dtypes._impl import fp32r as _fp32r_impl

B, C, T, H, W = 2, 512, 4, 32, 32
N = T * H * W            # 4096 tokens
NH = 8                   # heads
HD = C // NH             # 64
GROUPS = 32
EPS = 1e-5
NCORES = 8
HPC = 2                  # heads per core
KC = 32                  # key chunks of 128
KG = 3                   # key chunks per exp group (3 PSUM banks)

F32 = mybir.dt.float32
F32R = mybir.dt.float32r
BF16 = mybir.dt.bfloat16
MM_DT = BF16                      # dtype for all big matmul operands
AF = mybir.ActivationFunctionType
ALU = mybir.AluOpType

# AllGather groups = output-batch groups (core order defines row order)
AG_GROUPS = [[0, 1, 4, 5], [2, 3, 6, 7]]


def _round_fp32r(a: np.ndarray) -> np.ndarray:
    flat = np.ascontiguousarray(a, np.float32).view(np.uint32).ravel()
    r = _fp32r_impl.cast_fp32_to_fp32r(flat.size, flat)
    return np.asarray(r, np.uint32).reshape(a.shape).view(np.float32)


def build_nc(reps=1, skip_coll=False, skip_att=False, stages='full'):
    nc = bacc.Bacc(None, target_bir_lowering=False, debug=False,
                   num_devices=NCORES)

    xb = nc.dram_tensor("xb", [C, N], F32, kind="ExternalInput")
    gnw = nc.dram_tensor("gnw", [C], F32, kind="ExternalInput")
    gnb = nc.dram_tensor("gnb", [C], F32, kind="ExternalInput")
    maskc = nc.dram_tensor("maskc", [128, 4, 32], F32, kind="ExternalInput")
    mask32 = nc.dram_tensor("mask32", [32, C], F32, kind="ExternalInput")
    wq = nc.dram_tensor("wq", [C, 128], MM_DT, kind="ExternalInput")
    wk = nc.dram_tensor("wk", [C, 128], MM_DT, kind="ExternalInput")
    wv = nc.dram_tensor("wv", [C, 256], MM_DT, kind="ExternalInput")
    wo = nc.dram_tensor("wo", [C, 128], MM_DT, kind="ExternalInput")
    qb = nc.dram_tensor("qb", [128], F32, kind="ExternalInput")
    kb = nc.dram_tensor("kb", [128], F32, kind="ExternalInput")
    id66 = nc.dram_tensor("id66", [66, 66], MM_DT, kind="ExternalInput")
    obx = nc.dram_tensor("obx", [128, 128], F32, kind="ExternalInput")
    xres = nc.dram_tensor("xres", [128, N], F32, kind="ExternalInput")
    y = nc.dram_tensor("y", [128, N], F32, kind="ExternalOutput")

    with tile.TileContext(nc) as tc:
        for _ in range(reps):
            _body(nc, tc, xb, gnw, gnb, maskc, mask32, wq, wk, wv, wo,
                  qb, kb, id66, obx, xres, y,
                  skip_coll=skip_coll, skip_att=skip_att, stages=stages)
    nc.compile()
    return nc


def _body(nc, tc, xb, gnw, gnb, maskc, mask32, wq, wk, wv, wo,
          qb, kb, id66, obx, xres, y, skip_coll=False, skip_att=False,
          stages='full'):
    with (
        tc.tile_pool(name="const", bufs=1) as const,
        tc.tile_pool(name="dram", bufs=1, space="DRAM") as dram,
    ):
        # ---- constants / weights (DMA'd up front, overlap with GN) ----
        wq_sb = const.tile([128, 4, 128], MM_DT)
        wk_sb = const.tile([128, 4, 128], MM_DT)
        wv_sb = const.tile([128, 4, 256], MM_DT)
        wo_sb = const.tile([128, 4, 128], MM_DT)
        nc.sync.dma_start(out=wq_sb, in_=wq.rearrange("(c p) m -> p c m", p=128))
        nc.sync.dma_start(out=wk_sb, in_=wk.rearrange("(c p) m -> p c m", p=128))
        nc.sync.dma_start(out=wv_sb, in_=wv.rearrange("(c p) m -> p c m", p=128))
        nc.sync.dma_start(out=wo_sb, in_=wo.rearrange("(c p) m -> p c m", p=128))
        maskc_sb = const.tile([128, 4, 32], F32)
        mask32_sb = const.tile([32, C], F32)
        nc.sync.dma_start(out=maskc_sb, in_=maskc[:])
        nc.sync.dma_start(out=mask32_sb, in_=mask32[:])
        gnw_sb = const.tile([128, 4], F32)
        gnb_sb = const.tile([128, 4], F32)
        nc.sync.dma_start(out=gnw_sb, in_=gnw.rearrange("(t p) -> p t", p=128))
        nc.sync.dma_start(out=gnb_sb, in_=gnb.rearrange("(t p) -> p t", p=128))
        qb_sb = const.tile([128, 1], F32)
        kb_sb = const.tile([128, 1], F32)
        nc.sync.dma_start(out=qb_sb, in_=qb[:].unsqueeze(1))
        nc.sync.dma_start(out=kb_sb, in_=kb[:].unsqueeze(1))
        id66_sb = const.tile([66, 66], MM_DT)
        obx_sb = const.tile([128, 128], F32)
        nc.sync.dma_start(out=id66_sb, in_=id66[:])
        nc.sync.dma_start(out=obx_sb, in_=obx[:])
        eps_sb = const.tile([32, 1], F32)
        nc.vector.memset(eps_sb, EPS)
        zo_sb = const.tile([128, 2], F32)
        nc.vector.memset(zo_sb[:, 0:1], 1.0)
        nc.vector.memset(zo_sb[:, 1:2], 0.0)

        # DRAM bounce for the collective: flat layout of bounce_in is exactly
        # this core's 256 rows of the scrambled out-proj input.
        bounce_in = dram.tile([HPC, N, HD], MM_DT)
        bounce_out = dram.tile([4, HPC, N, HD], MM_DT)

        with tc.tile_pool(name="h", bufs=1) as h_pool:
            h_sb = [h_pool.tile([128, N], MM_DT, tag=f"h{t}", name=f"h{t}")
                    for t in range(4)]

            # ================= Phase A: GroupNorm =================
            with (
                tc.tile_pool(name="x", bufs=1) as x_pool,
                tc.tile_pool(name="gn_tmp", bufs=4) as gnt,
                tc.tile_pool(name="gn_ps", bufs=2, space="PSUM") as gn_ps,
            ):
                x_sb = []
                mv2 = []
                for t in range(4):
                    xt = x_pool.tile([128, N], F32, tag=f"x{t}")
                    nc.sync.dma_start(out=xt, in_=xb[128 * t:128 * (t + 1), :])
                    x_sb.append(xt)
                    stats = gnt.tile([128, 8, 6], F32, tag="stats")
                    for j in range(8):
                        nc.vector.bn_stats(out=stats[:, j, :],
                                           in_=xt[:, 512 * j:512 * (j + 1)])
                    mv = gnt.tile([128, 2], F32, tag="mv")
                    nc.vector.bn_aggr(out=mv, in_=stats)
                    # mv2 = (mean, E[x^2]) per channel
                    m2 = gnt.tile([128, 2], F32, tag=f"m2_{t}")
                    nc.vector.tensor_mul(m2[:, 0:1], mv[:, 0:1], mv[:, 0:1])
                    nc.vector.tensor_add(m2[:, 1:2], mv[:, 1:2], m2[:, 0:1])
                    nc.vector.tensor_copy(m2[:, 0:1], mv[:, 0:1])
                    mv2.append(m2)

                ps32 = gn_ps.tile([32, 2], F32, tag="ps32")
                for t in range(4):
                    nc.tensor.matmul(ps32, maskc_sb[:, t, :], mv2[t],
                                     start=(t == 0), stop=(t == 3))
                # group stats: (mean_g, rstd_g)  [32, 2]
                g_sb = gnt.tile([32, 2], F32, tag="g")
                ms = gnt.tile([32, 2], F32, tag="ms")
                nc.scalar.mul(ms, ps32, 1.0 / 16.0)       # (mean_g, E2_g)
                var = gnt.tile([32, 1], F32, tag="var")
                nc.vector.tensor_mul(var, ms[:, 0:1], ms[:, 0:1])
                nc.vector.tensor_sub(var, ms[:, 1:2], var)
                sd = gnt.tile([32, 1], F32, tag="sd")
                nc.scalar.activation(sd, var, AF.Sqrt, bias=eps_sb)
                nc.vector.tensor_copy(g_sb[:, 0:1], ms[:, 0:1])
                nc.vector.reciprocal(g_sb[:, 1:2], sd)

                for t in range(4):
                    psbc = gn_ps.tile([128, 2], F32, tag="psbc")
                    nc.tensor.matmul(psbc, mask32_sb[:, 128 * t:128 * (t + 1)],
                                     g_sb, start=True, stop=True)
                    sc = gnt.tile([128, 1], F32, tag="sc")
                    sh = gnt.tile([128, 1], F32, tag="sh")
                    nc.vector.tensor_mul(sc, psbc[:, 1:2], gnw_sb[:, t:t + 1])
                    nc.vector.tensor_mul(sh, psbc[:, 0:1], sc)
                    nc.vector.tensor_sub(sh, gnb_sb[:, t:t + 1], sh)
                    nc.vector.tensor_scalar(out=h_sb[t], in0=x_sb[t],
                                            scalar1=sc, scalar2=sh,
                                            op0=ALU.mult, op1=ALU.add)

            if stages == 'a':
                return
            # ================= Phase B: qkv projections =================
            with tc.tile_pool(name="kqv", bufs=1) as kqv:
                k_sb = kqv.tile([128, N], MM_DT, tag="k")
                q_sb = kqv.tile([128, N], MM_DT, tag="q")
                v_sb = [kqv.tile([128, HPC, HD + 2], MM_DT, tag=f"v{i}",
                                 name=f"v{i}")
                        for i in range(KC)]

                pps_cm = tc.tile_pool(name="proj_ps", bufs=2, space="PSUM")
                pps = pps_cm.__enter__()
                for t in range(8):
                    psq = pps.tile([128, 512], F32, tag="psq")
                    for cc in range(4):
                        nc.tensor.matmul(psq, wq_sb[:, cc, :],
                                         h_sb[cc][:, 512 * t:512 * (t + 1)],
                                         start=(cc == 0), stop=(cc == 3))
                    nc.vector.tensor_scalar(out=q_sb[:, 512 * t:512 * (t + 1)],
                                            in0=psq, scalar1=qb_sb,
                                            scalar2=None, op0=ALU.add)
                    psk = pps.tile([128, 512], F32, tag="psk")
                    for cc in range(4):
                        nc.tensor.matmul(psk, wk_sb[:, cc, :],
                                         h_sb[cc][:, 512 * t:512 * (t + 1)],
                                         start=(cc == 0), stop=(cc == 3))
                    nc.vector.tensor_scalar(out=k_sb[:, 512 * t:512 * (t + 1)],
                                            in0=psk, scalar1=kb_sb,
                                            scalar2=None, op0=ALU.add)

                for kt in range(KC):
                    psv = pps.tile([128, 256], F32, tag="psv")
                    for cc in range(4):
                        nc.tensor.matmul(psv,
                                         h_sb[cc][:, 128 * kt:128 * (kt + 1)],
                                         wv_sb[:, cc, :],
                                         start=(cc == 0), stop=(cc == 3))
                    # psv cols 0:64 head0, 64:128 head1 -> v_sb[kt][:, hi, 0:64]
                    nc.scalar.copy(
                        out=v_sb[kt][:, :, 0:HD],
                        in_=psv[:, 0:128].rearrange("p (h d) -> p h d", h=HPC))
                    for hi in range(HPC):
                        nc.vector.tensor_copy(v_sb[kt][:, hi, HD:HD + 2],
                                              zo_sb)
                pps_cm.__exit__(None, None, None)

                if skip_att or stages == 'ab':
                    return
                with tc.tile_pool(name="so", bufs=1) as so_pool:
                    so_sb = [so_pool.tile([HD + 2, 512], MM_DT,
                                          tag=f"so{i}", name=f"so{i}")
                             for i in range(HPC * 8)]

                    # ================= Phase C: attention =================
                    with (
                        tc.tile_pool(name="p_sb", bufs=2) as p_pool,
                        tc.tile_pool(name="s_ps", bufs=2, space="PSUM") as s_ps,
                        tc.tile_pool(name="o_ps", bufs=2, space="PSUM") as o_ps,
                    ):
                        groups = [(g * KG, min(KG, KC - g * KG))
                                  for g in range((KC + KG - 1) // KG)]
                        for hi in range(HPC):
                            hofs = HD * hi
                            for jq in range(8):
                                qsl = q_sb[hofs:hofs + HD,
                                           512 * jq:512 * (jq + 1)]
                                pso = o_ps.tile([HD + 2, 512], F32, tag="pso")
                                pending = None  # PV delayed one group so the
                                # PE FIFO never queues a PV (gated on ACT)
                                # ahead of the next S-group.
                                for g0, glen in groups:
                                    pss = s_ps.tile([128, KG * 512], F32,
                                                    tag="pss")
                                    for i in range(glen):
                                        kt = g0 + i
                                        nc.tensor.matmul(
                                            pss[:, 512 * i:512 * (i + 1)],
                                            k_sb[hofs:hofs + HD,
                                                 128 * kt:128 * (kt + 1)],
                                            qsl, start=True, stop=True)
                                    pt = p_pool.tile([128, KG * 512], MM_DT,
                                                     tag="pt")
                                    nc.scalar.activation(
                                        pt[:, 0:512 * glen],
                                        pss[:, 0:512 * glen],
                                        AF.Exp, scale=float(1.0 / np.sqrt(HD)))
                                    if pending is not None:
                                        pg0, pglen, ppt = pending
                                        for i in range(pglen):
                                            kt = pg0 + i
                                            nc.tensor.matmul(
                                                pso, v_sb[kt][:, hi, :],
                                                ppt[:, 512 * i:512 * (i + 1)],
                                                start=(kt == 0),
                                                stop=(kt == KC - 1))
                                    pending = (g0, glen, pt)
                                pg0, pglen, ppt = pending
                                for i in range(pglen):
                                    kt = pg0 + i
                                    nc.tensor.matmul(
                                        pso, v_sb[kt][:, hi, :],
                                        ppt[:, 512 * i:512 * (i + 1)],
                                        start=(kt == 0), stop=(kt == KC - 1))
                                nc.vector.tensor_copy(so_sb[8 * hi + jq], pso)

                    if stages == 'abc':
                        return
                    # ====== Phase C2: transpose to token-major + normalize ===
                    with (
                        tc.tile_pool(name="otok", bufs=3) as otok_pool,
                        tc.tile_pool(name="c2_tmp", bufs=8) as c2t,
                        tc.tile_pool(name="t_ps", bufs=4, space="PSUM") as t_ps,
                    ):
                        for hi in range(HPC):
                            for jq in range(8):
                                so = so_sb[8 * hi + jq]
                                ot = otok_pool.tile([128, 4, HD], MM_DT,
                                                    tag="ot")
                                for cpos in range(4):
                                    tps = t_ps.tile([128, HD + 2], MM_DT,
                                                    tag="tps")
                                    nc.tensor.transpose(
                                        tps, so[:, 128 * cpos:128 * (cpos + 1)],
                                        id66_sb)
                                    recip = c2t.tile([128, 1], F32, tag="recip")
                                    nc.vector.reciprocal(recip,
                                                         tps[:, HD:HD + 1])
                                    nc.vector.tensor_scalar(
                                        out=ot[:, cpos, :],
                                        in0=tps[:, 0:HD],
                                        scalar1=recip, scalar2=None,
                                        op0=ALU.mult)
                                nc.sync.dma_start(
                                    out=bounce_in[hi,
                                                  512 * jq:512 * (jq + 1), :]
                                    .rearrange("(c r) d -> r c d", c=4),
                                    in_=ot)

        # ============ Phase D: AllGather + scrambled out projection ==========
        if stages == 'abcc2':
            return
        if not skip_coll:
            nc.gpsimd.collective_compute(
                "AllGather", ALU.bypass,
                replica_groups=AG_GROUPS,
                ins=[bounce_in.opt()],
                outs=[bounce_out.opt()],
            )
        with (
            tc.tile_pool(name="og", bufs=3) as og_pool,
            tc.tile_pool(name="fin", bufs=4) as fin,
            tc.tile_pool(name="xr", bufs=1) as xr_pool,
            tc.tile_pool(name="d_ps", bufs=4, space="PSUM") as d_ps,
        ):
            xr_sb = xr_pool.tile([128, N], F32)
            nc.sync.dma_start(out=xr_sb, in_=xres[:])
            xr_v = xr_sb.rearrange("p (j q) -> p j q", q=128)
            y_v = y.rearrange("p (j q) -> p j q", q=128)
            for b in range(2):
                # scram chunk cc rows = head (rank-pair 2b + cc//2, local cc%2)
                srcs = [bounce_out[2 * b + cc // 2, cc % 2]
                        .rearrange("(nh5 j) d -> nh5 (j d)", j=32)
                        for cc in range(4)]
                obx_b = obx_sb[:, 64 * b:64 * (b + 1)]
                for tt in range(4):
                    og = og_pool.tile([128, 4, 512], MM_DT, tag="og")
                    for cc in range(4):
                        nc.sync.dma_start(
                            out=og[:, cc, :],
                            in_=srcs[cc][:, 512 * tt:512 * (tt + 1)])
                    psd = d_ps.tile([128, 512], F32, tag="psd")
                    for cc in range(4):
                        nc.tensor.matmul(psd, wo_sb[:, cc, :], og[:, cc, :],
                                         start=(cc == 0), stop=(cc == 3))
                    psd_v = psd.rearrange("p (j d) -> p j d", d=HD)
                    t1 = fin.tile([128, 8, HD], F32, tag="t1")
                    nc.vector.tensor_add(
                        t1, psd_v,
                        obx_b.unsqueeze(1).broadcast_to((128, 8, HD)))
                    out_sb = fin.tile([128, 8, HD], F32, tag="out")
                    nc.vector.tensor_add(
                        out_sb, t1,
                        xr_v[:, 8 * tt:8 * (tt + 1), 64 * b:64 * (b + 1)])
                    nc.sync.dma_start(
                        out=y_v[:, 8 * tt:8 * (tt + 1), 64 * b:64 * (b + 1)],
                        in_=out_sb)


# =========================== host-side driver ===========================

def prep_in_maps(x, gn_w, gn_b, qkv_w, qkv_b, out_w, out_b):
    """Build the 8 per-core input maps from the full (unsharded) inputs."""
    x = np.asarray(x, np.float32)
    gn_w = np.asarray(gn_w, np.float32)
    gn_b = np.asarray(gn_b, np.float32)
    qkv_w = np.asarray(qkv_w, np.float32)
    qkv_b = np.asarray(qkv_b, np.float32)
    out_w = np.asarray(out_w, np.float32)
    out_b = np.asarray(out_b, np.float32)

    xf = x.reshape(B, C, N)
    maskc = np.zeros((128, 4, 32), np.float32)
    for t in range(4):
        for p in range(128):
            maskc[p, t, 8 * t + p // 16] = 1.0
    mask32 = np.zeros((32, C), np.float32)
    for c in range(C):
        mask32[c // 16, c] = 1.0
    id66 = np.eye(66, dtype=np.float32)

    qkv_wr = qkv_w.reshape(3, NH, HD, C)
    qkv_br = qkv_b.reshape(3, NH, HD)
    vb_full = qkv_br[2]                      # [NH, HD]
    # position of each core inside its AllGather group + group id (= b')
    pos = {}
    grp = {}
    for gi, cores in enumerate(AG_GROUPS):
        for p, cid in enumerate(cores):
            pos[cid] = p
            grp[cid] = gi

    in_maps = []
    for cid in range(NCORES):
        b = cid // 4               # batch this core attends over
        h0 = HPC * (cid % 4)       # first head this core computes
        bg = grp[cid]              # output-batch group for phase D
        p = pos[cid]               # output-channel slice for phase D
        wq_c = qkv_wr[0, h0:h0 + HPC].reshape(128, C).T     # [C, 128]
        wk_c = qkv_wr[1, h0:h0 + HPC].reshape(128, C).T
        wv_c = qkv_wr[2, h0:h0 + HPC].reshape(128, C).T
        wv_pad = np.zeros((C, 256), np.float32)
        wv_pad[:, :128] = wv_c
        oc = slice(128 * p, 128 * (p + 1))
        # obx[ocl, b*64 + d] = out_b[oc] + sum_hm (sum_nh5 w_o[oc, hm*128+nh5])
        #                                  * vb[4*bg + hm, d]
        w_oc = out_w[oc]                                     # [128, 512]
        wsum = w_oc.reshape(128, 4, 128).sum(axis=2)         # [128, 4] per hm
        vbg = vb_full[4 * bg:4 * bg + 4]                     # [4, HD]
        add = wsum @ vbg                                     # [128, HD]
        obx = np.zeros((128, 128), np.float32)
        for bb in range(2):
            obx[:, 64 * bb:64 * (bb + 1)] = out_b[oc][:, None] + add
        in_maps.append({
            "xb": np.ascontiguousarray(xf[b]),
            "gnw": gn_w, "gnb": gn_b,
            "maskc": maskc, "mask32": mask32,
            "wq": wq_c.astype(ml_dtypes.bfloat16),
            "wk": wk_c.astype(ml_dtypes.bfloat16),
            "wv": wv_pad.astype(ml_dtypes.bfloat16),
            "wo": np.ascontiguousarray(w_oc.T).astype(ml_dtypes.bfloat16),
            "qb": np.ascontiguousarray(qkv_br[0, h0:h0 + HPC].reshape(128)),
            "kb": np.ascontiguousarray(qkv_br[1, h0:h0 + HPC].reshape(128)),
            "id66": id66.astype(ml_dtypes.bfloat16),
            "obx": obx,
            "xres": np.ascontiguousarray(xf[bg, oc, :]),
        })
    return in_maps


def assemble_output(results):
    y = np.empty((B, C, N), np.float32)
    for gi, cores in enumerate(AG_GROUPS):
        for p, cid in enumerate(cores):
            y[gi, 128 * p:128 * (p + 1), :] = results[cid]["y"]
    return y.reshape(B, C, T, H, W)


_NC_CACHE = None


def get_nc():
    global _NC_CACHE
    if _NC_CACHE is None:
        _NC_CACHE = build_nc()
    return _NC_CACHE


def kernel(x, gn_w, gn_b, qkv_w, qkv_b, out_w, out_b):
    in_maps = prep_in_maps(x, gn_w, gn_b, qkv_w, qkv_b, out_w, out_b)
    nc = get_nc()
    res = run_bass_kernel_spmd(nc, in_maps, core_ids=list(range(NCORES)))
    out = assemble_output(res.results)
    return out.astype(np.asarray(x).dtype, copy=False)
